# revision 2
# baseline (speedup 1.0000x reference)
"""Masked single-head attention on 8 TRN2 NeuronCores.

Problem: q,k,v [8, 2048, 128] f32, event_lengths [8] i32.
  scores = q @ k^T / sqrt(128); masked (i<len & j<len) else -1e9; softmax; @ v.

Sharding: data-parallel over batch — core b computes batch element b's full
2048x2048 attention.

Per-core algorithm (transposed-scores layout, no max-subtraction):
  S^T[j,i] = sum_d kT[d,j] * qT[d,i]            (f32r matmuls, N=512)
  E[j,i]   = exp(S^T/sqrt(128) + akv[j])        (ACT; akv[j] = 0 / -1e9 kv mask,
                                                 applied free as per-partition bias)
  o^T[d,i] = sum_j v[j,d] * E[j,i]              (f32r matmuls, accumulated in PSUM)
  sum[i]   = sum_j E[j,i]                       (ones-vector matmul)
  out[i,d] = o^T.T * (mq[i]/sum[i]) + meanV[d] * (1-mq[i])
             (PE transpose + per-partition scalars; mq[i] is the query mask,
              rows i>=len get mean(V) exactly like the reference's uniform
              softmax over an all -1e9 row)

Scores are bounded (|s| < ~20) so exp() cannot overflow in f32 and skipping the
max-subtraction is exact up to normal fp rounding.
"""

import numpy as np

HEAD_DIM = 128
B, S = 8, 2048
NEG = -1e9
P = 128
NCHUNK = S // P  # 16 j-chunks
BLKW = 512  # query block width
NBLK = S // BLKW  # 4
INV_SQRT_D = 1.0 / np.sqrt(HEAD_DIM)
EPS = 1e-30

_cache = {}
_last_in_maps = None


def _build():
    import concourse.tile as tile
    from concourse import bacc, mybir

    F32 = mybir.dt.float32
    F32R = mybir.dt.float32r
    EXP = mybir.ActivationFunctionType.Exp

    nc = bacc.Bacc("TRN2", target_bir_lowering=False, debug=False, num_devices=8)

    qT = nc.declare_dram_parameter("qT", [P, S], F32R, isOutput=False)
    kT = nc.declare_dram_parameter("kT", [P, S], F32R, isOutput=False)
    v = nc.declare_dram_parameter("v", [S, P], F32R, isOutput=False)
    akvT = nc.declare_dram_parameter("akvT", [P, NCHUNK], F32, isOutput=False)
    mqT = nc.declare_dram_parameter("mqT", [P, NBLK * 4], F32, isOutput=False)
    nmqT = nc.declare_dram_parameter("nmqT", [P, NBLK * 4], F32, isOutput=False)
    onesc = nc.declare_dram_parameter("onesc", [P, 1], F32R, isOutput=False)
    meanvb = nc.declare_dram_parameter("meanvb", [P, P], F32, isOutput=False)
    identd = nc.declare_dram_parameter("identd", [P, P], F32, isOutput=False)
    out = nc.declare_dram_parameter("out", [S, P], F32, isOutput=True)

    with tile.TileContext(nc) as tc:
        with (
            tc.tile_pool(name="const", bufs=1) as const,
            tc.tile_pool(name="qk", bufs=1) as qk,
            tc.tile_pool(name="vp", bufs=1) as vp,
            tc.tile_pool(name="e", bufs=6) as epool,
            tc.tile_pool(name="stage", bufs=3) as stage,
            tc.tile_pool(name="ps_s", bufs=3, space="PSUM") as ps_s,
            tc.tile_pool(name="ps_o", bufs=2, space="PSUM") as ps_o,
            tc.tile_pool(name="ps_sum", bufs=2, space="PSUM") as ps_sum,
            tc.tile_pool(name="ps_t", bufs=1, space="PSUM") as ps_t,
        ):
            # ---- constants / resident inputs ----
            akv_t = const.tile([P, NCHUNK], F32, tag="akv")
            nc.sync.dma_start(akv_t[:], akvT[:, :])
            mq_t = const.tile([P, NBLK * 4], F32, tag="mq")
            nc.sync.dma_start(mq_t[:], mqT[:, :])
            nmq_t = const.tile([P, NBLK * 4], F32, tag="nmq")
            nc.sync.dma_start(nmq_t[:], nmqT[:, :])
            ones_t = const.tile([P, 1], F32R, tag="ones")
            nc.sync.dma_start(ones_t[:], onesc[:, :])
            meanv_t = const.tile([P, P], F32, tag="meanv")
            nc.sync.dma_start(meanv_t[:], meanvb[:, :])
            ident_t = const.tile([P, P], F32, tag="ident")
            nc.sync.dma_start(ident_t[:], identd[:, :])

            # kT resident, loaded per chunk so the first matmuls start early
            kT_t = qk.tile([P, S], F32R, tag="kT")
            for jc in range(NCHUNK):
                nc.sync.dma_start(kT_t[:, jc * P : (jc + 1) * P], kT[:, jc * P : (jc + 1) * P])
            # qT resident, loaded per block
            qT_t = qk.tile([P, S], F32R, tag="qT")
            for ib in range(NBLK):
                nc.sync.dma_start(
                    qT_t[:, ib * BLKW : (ib + 1) * BLKW], qT[:, ib * BLKW : (ib + 1) * BLKW]
                )
            # v resident as 16 chunks [128, 128]
            v_t = vp.tile([P, NCHUNK * P], F32R, tag="v")
            for jc in range(NCHUNK):
                nc.sync.dma_start(v_t[:, jc * P : (jc + 1) * P], v[jc * P : (jc + 1) * P, :])

            for ib in range(NBLK):
                qslice = qT_t[:, ib * BLKW : (ib + 1) * BLKW]
                po = ps_o.tile([P, BLKW], F32, tag="o")
                psm = ps_sum.tile([1, BLKW], F32, tag="sum")
                for jc in range(NCHUNK):
                    # scores^T chunk [j=128, i=512]
                    ps = ps_s.tile([P, BLKW], F32, tag="s")
                    nc.tensor.matmul(
                        ps[:], kT_t[:, jc * P : (jc + 1) * P], qslice, start=True, stop=True
                    )
                    # E = exp(s/sqrt(d) + akv[j])  (kv mask via per-partition bias)
                    e = epool.tile([P, BLKW], F32R, tag="e")
                    nc.scalar.activation(
                        e[:], ps[:], EXP, bias=akv_t[:, jc : jc + 1], scale=INV_SQRT_D
                    )
                    # o^T += v_chunk.T @ E ; sum += ones.T @ E
                    nc.tensor.matmul(
                        po[:], v_t[:, jc * P : (jc + 1) * P], e[:],
                        start=(jc == 0), stop=(jc == NCHUNK - 1),
                    )
                    nc.tensor.matmul(
                        psm[:], ones_t[:], e[:],
                        start=(jc == 0), stop=(jc == NCHUNK - 1),
                    )

                # copy o^T and sums to SBUF
                ot = stage.tile([P, BLKW], F32, tag="ot")
                nc.vector.tensor_copy(ot[:], po[:])
                sums = stage.tile([1, BLKW], F32, tag="sums")
                nc.vector.tensor_copy(sums[:], psm[:])

                # transpose sums [1,512] -> [128,4] via 4 tiny matmuls with [[1.0]]
                prs = ps_sum.tile([P, 4], F32, tag="sum")
                for c in range(4):
                    nc.tensor.matmul(
                        prs[:, c : c + 1],
                        sums[0:1, c * P : (c + 1) * P],
                        ident_t[0:1, 0:1],
                        start=True, stop=True,
                    )
                # r = 1 / (sums + eps), then a = r * mq (both [128,4])
                rs = stage.tile([P, 4], F32, tag="rs")
                nc.vector.tensor_scalar(rs[:], prs[:], EPS, None, mybir.AluOpType.add)
                nc.vector.reciprocal(rs[:], rs[:])
                a_t = stage.tile([P, 4], F32, tag="a")
                nc.vector.tensor_tensor(
                    a_t[:], rs[:], mq_t[:, ib * 4 : (ib + 1) * 4], mybir.AluOpType.mult
                )

                # transpose o^T blocks to [i,d], normalize, blend meanV, DMA out
                pt = ps_t.tile([P, BLKW], F32, tag="t")
                for c in range(4):
                    nc.tensor.transpose(
                        pt[:, c * P : (c + 1) * P], ot[:, c * P : (c + 1) * P], ident_t[:]
                    )
                for c in range(4):
                    col = ib * 4 + c
                    fin = stage.tile([P, P], F32, tag="fin")
                    # fin = o[i,d] * (mq[i]/sum[i])
                    nc.vector.tensor_scalar(
                        fin[:], pt[:, c * P : (c + 1) * P],
                        a_t[:, c : c + 1], None, mybir.AluOpType.mult,
                    )
                    # fin += meanV_bcast * (1-mq[i])
                    mterm = stage.tile([P, P], F32, tag="mterm")
                    nc.vector.tensor_scalar(
                        mterm[:], meanv_t[:],
                        nmq_t[:, col : col + 1], None, mybir.AluOpType.mult,
                    )
                    nc.vector.tensor_tensor(
                        fin[:], fin[:], mterm[:], mybir.AluOpType.add
                    )
                    nc.sync.dma_start(out[col * P : (col + 1) * P, :], fin[:])

    nc.compile()
    return nc


def _get_nc():
    if "nc" not in _cache:
        _cache["nc"] = _build()
    return _cache["nc"]


def kernel(q, k, v, event_lengths):
    q = np.asarray(q, dtype=np.float32)
    k = np.asarray(k, dtype=np.float32)
    v = np.asarray(v, dtype=np.float32)
    lens = np.asarray(event_lengths).astype(np.int64)

    nc = _get_nc()

    ident = np.eye(P, dtype=np.float32)
    onesc = np.ones((P, 1), np.float32)
    j_idx = np.arange(S)

    in_maps = []
    for b in range(B):
        ln = int(lens[b])
        akv = np.where(j_idx < ln, 0.0, NEG).astype(np.float32)
        mq = (j_idx < ln).astype(np.float32)
        in_maps.append(
            {
                "qT": np.ascontiguousarray(q[b].T),
                "kT": np.ascontiguousarray(k[b].T),
                "v": v[b],
                "akvT": np.ascontiguousarray(akv.reshape(NCHUNK, P).T),
                "mqT": np.ascontiguousarray(mq.reshape(NBLK * 4, P).T),
                "nmqT": np.ascontiguousarray((1.0 - mq).reshape(NBLK * 4, P).T),
                "onesc": onesc,
                "meanvb": np.broadcast_to(
                    v[b].mean(axis=0, dtype=np.float64).astype(np.float32), (P, P)
                ).copy(),
                "identd": ident,
            }
        )

    from concourse.bass_utils import run_bass_kernel_spmd

    global _last_in_maps
    _last_in_maps = in_maps
    res = run_bass_kernel_spmd(nc, in_maps, core_ids=list(range(B)))
    out = np.stack([res.results[b]["out"] for b in range(B)], axis=0)
    return out


if __name__ == "__main__":
    rng = np.random.default_rng(0)
    q = rng.standard_normal((B, S, HEAD_DIM)).astype(np.float32)
    k = rng.standard_normal((B, S, HEAD_DIM)).astype(np.float32)
    v_ = rng.standard_normal((B, S, HEAD_DIM)).astype(np.float32)
    lens = rng.integers(0, S, size=(B,)).astype(np.int32)
    o = kernel(q=q, k=k, v=v_, event_lengths=lens)
    print(o.shape, o.dtype)
